# revision 1
# baseline (speedup 1.0000x reference)
"""BCH/RS systematic encoder kernel for Trainium2 (8 NeuronCores, data parallel).

Computes out = concat([msg, (msg @ Gp) mod 2], axis=-1) for
msg [16384, 1000] f32 of 0/1 bits and Gp [1000, 256] f32 of 0/1 bits.

Design (per core, 2048 rows, 16 chunks of 128):
  - SWDGE cast-load msg chunk f32 -> bf16 SBUF (0/1 exact in bf16)
  - SWDGE cast-store bf16 -> f32 to out[:, :1000] (systematic copy-through)
  - DMA xbar transpose (2-byte) 128x128 blocks: msg natural -> msgT [k, m]
  - 8 accumulating bf16 matmuls: psum[m,256] += msgT_k.T @ Gp_k (fp32 accum, exact)
  - DVE tensor_scalar mod 2.0 on psum -> SBUF f32
  - store parity to out[:, 1000:1256]
HBM traffic/core = 8.19 MB read + 10.29 MB write (the minimum).
"""

import os
import sys

import numpy as np

if os.path.isdir("/opt/trn_rl_repo") and "/opt/trn_rl_repo" not in sys.path:
    sys.path.insert(0, "/opt/trn_rl_repo")

import ml_dtypes

import concourse.bacc as bacc
import concourse.mybir as mybir
import concourse.tile as tile
from concourse.bass_utils import run_bass_kernel_spmd

BATCH = 16384
MSG = 1000
NPAR = 256
NCORES = 8
ROWS = BATCH // NCORES  # 2048
P = 128
KCH = 8  # k chunks; padded K = 1024
KPAD = KCH * P

# test.py pokes these for profiling
TRACE = False
LAST_RESULT = None

_CACHE = {}


def build_nc(rows=ROWS):
    """Emit the Bass/Tile IR for one core handling `rows` rows."""
    mch = rows // P
    nc = bacc.Bacc("TRN2", target_bir_lowering=False, debug=False)
    msg = nc.dram_tensor("msg", [rows, MSG], mybir.dt.float32, kind="ExternalInput")
    gp = nc.dram_tensor("gp", [P, KCH * NPAR], mybir.dt.bfloat16, kind="ExternalInput")
    out = nc.dram_tensor(
        "out", [rows, MSG + NPAR], mybir.dt.float32, kind="ExternalOutput"
    )

    SC = 2  # m-chunks per superchunk (SWDGE/DVE batching granularity)
    n_super = mch // SC
    LAG = 1  # stores trail compute by this many superchunks
    msg3 = msg[:, :].rearrange("(s c p) k -> s c p k", c=SC, p=P)
    out3 = out[:, :].rearrange("(s c p) k -> s c p k", c=SC, p=P)

    with tile.TileContext(nc) as tc:
        with (
            tc.tile_pool(name="gpool", bufs=1) as gpool,
            # every superchunk's a-tile is resident at once: loads all run
            # upfront and never wait on a recycled slot (or a store's SWDGE
            # semaphore lane)
            tc.tile_pool(name="apool", bufs=n_super + 1) as apool,
            tc.tile_pool(name="bpool", bufs=6) as bpool,
            tc.tile_pool(name="cpool", bufs=4) as cpool,
            tc.tile_pool(name="epool", bufs=4) as epool,
            tc.tile_pool(name="ppool", bufs=8, space="PSUM") as ppool,
        ):
            # Gp resident in SBUF: gsb[q, kb*256 + n] = Gp_padded[kb*128 + q, n]
            gsb = gpool.tile([P, KCH * NPAR], mybir.dt.bfloat16)
            nc.sync.dma_start(out=gsb[:, :], in_=gp[:, :])

            a_tiles = {}
            es = {}

            # row stride must keep every a[:, c, :] slice 32B-aligned for the
            # xbar transpose: 1264 bf16 = 2528 B = 79*32
            ROWP = 1264

            def emit_load(si):
                # full output row in bf16: cols 0:1000 msg, 1000:1256 parity.
                # No zero-pad memset: the last k-chunk matmul contracts K=104,
                # so the PE never reads the transposed garbage rows.
                a = apool.tile([P, SC, ROWP], mybir.dt.bfloat16, tag="a")
                nc.gpsimd.dma_start(
                    out=a[:, :, 0:MSG], in_=msg3[si, :, :, :].rearrange("c p k -> p c k")
                )
                a_tiles[si] = a

            def emit_compute(si):
                a = a_tiles[si]
                # per-chunk xbar transpose: b[q, c*KCH + kb, p] = a[p, c, kb*128+q]
                # all on ONE HWDGE ring: concurrent xbar transposes from two
                # rings corrupt each other (shared xbar; this Tile does not
                # cross-engine-serialize them)
                b = bpool.tile([P, SC * KCH, P], mybir.dt.bfloat16, tag="b")
                for c in range(SC):
                    nc.sync.dma_start(
                        out=b[:, c * KCH : (c + 1) * KCH, :],
                        in_=a[:, c, 0:KPAD],
                        transpose=True,
                    )
                # both chunks accumulate side by side in one PSUM bank
                acc = ppool.tile([P, SC * NPAR], mybir.dt.float32, tag="acc")
                for c in range(SC):
                    for kb in range(KCH):
                        kk = P if kb < KCH - 1 else MSG - (KCH - 1) * P  # 104 tail
                        nc.tensor.matmul(
                            acc[:, c * NPAR : (c + 1) * NPAR],
                            b[0:kk, c * KCH + kb, :],
                            gsb[0:kk, kb * NPAR : (kb + 1) * NPAR],
                            start=(kb == 0),
                            stop=(kb == KCH - 1),
                        )
                # exact-integer f32 -> i32 eviction in ONE op on idle ACT
                c_i32 = cpool.tile([P, SC, NPAR], mybir.dt.int32, tag="c")
                nc.scalar.copy(
                    c_i32[:, :, :].rearrange("p c n -> p (c n)"), acc[:, :]
                )
                # mod 2 == AND 1 (bitVec op cannot cast, keep i32)
                e = epool.tile([P, SC, NPAR], mybir.dt.int32, tag="e")
                nc.vector.tensor_scalar(
                    e[:, :, :], c_i32[:, :, :], 1, None, mybir.AluOpType.bitwise_and
                )
                # parity into the output-row tile (0/1 exact in bf16)
                nc.vector.tensor_copy(a[:, :, MSG : MSG + NPAR], e[:, :, :])

            def emit_store(si):
                # single cast-store of the full rows: [p, c, 1256] bf16 -> f32
                a = a_tiles.pop(si)
                nc.gpsimd.dma_start(
                    out=out3[si, :, :, :].rearrange("c p k -> p c k"),
                    in_=a[:, :, 0 : MSG + NPAR],
                )

            for it in range(n_super):
                emit_load(it)
            # zero the transpose pad columns once per (fresh) slot, batched on
            # DVE before the compute chain (keeps CoreSim's uninit checker
            # happy; PE never reads those rows thanks to the K=104 tail)
            for it in range(n_super):
                nc.vector.memset(a_tiles[it][:, :, MSG:KPAD], 0)
            for it in range(n_super + LAG):
                if it < n_super:
                    emit_compute(it)
                k = it - LAG
                if 0 <= k < n_super:
                    emit_store(k)

    nc.compile()
    return nc


def prep_gp(Gp):
    """Pad Gp to 1024 rows and swizzle to the [128, 8*256] bf16 SBUF layout."""
    gp = np.asarray(Gp, dtype=np.float32)
    gp_pad = np.zeros((KPAD, NPAR), dtype=np.float32)
    gp_pad[:MSG] = gp
    gsw = gp_pad.reshape(KCH, P, NPAR).transpose(1, 0, 2).reshape(P, KCH * NPAR)
    return np.ascontiguousarray(gsw).astype(ml_dtypes.bfloat16)


def kernel(message_bits, Gp):
    global LAST_RESULT
    msg = np.ascontiguousarray(np.asarray(message_bits, dtype=np.float32))
    assert msg.shape == (BATCH, MSG), msg.shape
    gsw = prep_gp(Gp)

    if "nc" not in _CACHE:
        _CACHE["nc"] = build_nc()
    nc = _CACHE["nc"]

    in_maps = [
        {"msg": msg[i * ROWS : (i + 1) * ROWS], "gp": gsw} for i in range(NCORES)
    ]
    res = run_bass_kernel_spmd(
        nc, in_maps, core_ids=list(range(NCORES)), trace=TRACE
    )
    LAST_RESULT = res
    return np.concatenate([r["out"] for r in res.results], axis=0)



# revision 3
# speedup vs baseline: 2.2795x; 2.2795x over previous
"""BCH/RS systematic encoder kernel for Trainium2 (8 NeuronCores, data parallel).

Computes out = concat([msg, (msg @ Gp) mod 2], axis=-1) for
msg [16384, 1000] f32 of 0/1 bits and Gp [1000, 256] f32 of 0/1 bits.

Design (per core, 2048 rows, 4 groups of 512 = 4 chunks of 128):
  - SWDGE cast-load msg group f32 -> fp16 SBUF (0/1 exact in fp16)
  - transpose msg [m,k] -> msgT [k,m] per 128x128 block, either on the PE
    (matmul-with-identity into PSUM fp16, ACT/DVE eviction) or the DMA xbar
  - Gp column-packed: Gp2[k,n'] = Gp[k,n'] + 1024*Gp[k,n'+128] (fp16 exact,
    values {0,1,1024,1025}); stationary operand [k=128, n'=128] per k-chunk
  - 8 accumulating matmuls per group: S[n'=128, m=512] += Gp2_kb.T @ msgT_kb
    (f32 PSUM; S <= 1000*1025 < 2^24 so every partial sum is exact)
  - S holds BOTH parity halves: lo = S & 1, hi = (S >> 10) & 1
  - PE f32 transpose of S -> [m, n'], ACT psum->i32, DVE bit extracts,
    cast-copies into the fp16 row tile cols 1000:1128 / 1128:1256
  - SWDGE cast-store fp16 rows -> f32 out [m, 1256]
HBM traffic/core = 8.19 MB read + 10.29 MB write (the minimum).
"""

import os
import sys

import numpy as np

if os.path.isdir("/opt/trn_rl_repo") and "/opt/trn_rl_repo" not in sys.path:
    sys.path.insert(0, "/opt/trn_rl_repo")

import concourse.bacc as bacc
import concourse.masks as masks
import concourse.mybir as mybir
import concourse.tile as tile
from concourse.bass_utils import run_bass_kernel_spmd

BATCH = 16384
MSG = 1000
NPAR = 256
NPACK = 128  # packed parity columns (two bits per matmul output value)
NCORES = 8
ROWS = BATCH // NCORES  # 2048
P = 128
KCH = 8  # k chunks; padded K = 1024
KPAD = KCH * P
GC = 4  # m-chunks of 128 per group
GM = GC * P  # 512 rows per group

# 'pe': msg transpose on the tensor engine; 'xbar': on the DMA crossbar
TMODE = "pe"

# test.py pokes these for profiling
TRACE = False
LAST_RESULT = None

_CACHE = {}


def build_nc(rows=ROWS, tmode=None):
    """Emit the Bass/Tile IR for one core handling `rows` rows."""
    tmode = tmode or TMODE
    n_groups = rows // GM
    assert rows == n_groups * GM
    nc = bacc.Bacc("TRN2", target_bir_lowering=False, debug=False)
    msg = nc.dram_tensor("msg", [rows, MSG], mybir.dt.float32, kind="ExternalInput")
    gp = nc.dram_tensor("gp", [P, KCH * NPACK], mybir.dt.float16, kind="ExternalInput")
    out = nc.dram_tensor(
        "out", [rows, MSG + NPAR], mybir.dt.float32, kind="ExternalOutput"
    )

    msg3 = msg[:, :].rearrange("(g c p) k -> g c p k", c=GC, p=P)
    out3 = out[:, :].rearrange("(g c p) k -> g c p k", c=GC, p=P)

    # row stride keeps every a[:, c, :] slice 32B-aligned for the xbar
    # transpose: 1264 fp16 = 2528 B = 79*32
    ROWP = 1264

    with tile.TileContext(nc) as tc:
        with (
            tc.tile_pool(name="gpool", bufs=1) as gpool,
            tc.tile_pool(name="ipool", bufs=1) as ipool,
            tc.tile_pool(name="apool", bufs=min(n_groups, 4)) as apool,
            tc.tile_pool(name="bpool", bufs=2) as bpool,
            tc.tile_pool(name="sevpool", bufs=2) as sevpool,
            tc.tile_pool(name="cipool", bufs=2) as cipool,
            tc.tile_pool(name="epool", bufs=2) as epool,
            tc.tile_pool(name="ptpool", bufs=3, space="PSUM") as ptpool,
            tc.tile_pool(name="accpool", bufs=3, space="PSUM") as accpool,
            tc.tile_pool(name="stpool", bufs=2, space="PSUM") as stpool,
        ):
            # Gp packed+swizzled resident in SBUF: gsb[q, kb, n'] = Gp2[kb*128+q, n']
            gsb = gpool.tile([P, KCH, NPACK], mybir.dt.float16)
            nc.sync.dma_start(
                out=gsb[:, :, :].rearrange("p a b -> p (a b)"), in_=gp[:, :]
            )
            ident16 = ipool.tile([P, P], mybir.dt.float16, tag="i16")
            masks.make_identity(nc, ident16[:, :])
            ident32 = ipool.tile([P, P], mybir.dt.float32, tag="i32")
            masks.make_identity(nc, ident32[:, :])

            a_tiles = {}
            b_tiles = {}
            acc_tiles = {}

            def emit_load(g):
                # full output row in fp16: cols 0:1000 msg, 1000:1256 parity
                a = apool.tile([P, GC, ROWP], mybir.dt.float16, tag="a")
                nc.gpsimd.dma_start(
                    out=a[:, :, 0:MSG],
                    in_=msg3[g, :, :, :].rearrange("c p k -> p c k"),
                )
                # zero the transpose pad columns (the pad rows of msgT hit
                # zero Gp2 rows / are zeros streaming into the matmul)
                nc.vector.memset(a[:, :, MSG:KPAD], 0)
                a_tiles[g] = a

            def emit_transpose(g):
                a = a_tiles[g]
                # b[q, c, kb, p] = a[p, c, kb*128+q]  (msgT, k on partitions)
                b = bpool.tile([P, GC, KCH, P], mybir.dt.float16, tag="b")
                if tmode == "xbar":
                    # all on ONE HWDGE ring: concurrent xbar transposes from
                    # two rings corrupt each other
                    for c in range(GC):
                        nc.sync.dma_start(
                            out=b[:, c, :, :], in_=a[:, c, 0:KPAD], transpose=True
                        )
                else:
                    for c in range(GC):
                        pt = ptpool.tile([P, KCH, P], mybir.dt.float16, tag="pt")
                        for kb in range(KCH):
                            nc.tensor.transpose(
                                pt[:, kb, :],
                                a[:, c, kb * P : (kb + 1) * P],
                                ident16[:, :],
                            )
                        # evict msgT chunk PSUM -> SBUF (split ACT/DVE)
                        if c % 2 == 0:
                            nc.scalar.copy(b[:, c, :, :], pt[:, :, :])
                        else:
                            nc.vector.tensor_copy(b[:, c, :, :], pt[:, :, :])
                b_tiles[g] = b

            def emit_mm(g):
                b = b_tiles[g]
                # S[n'=128, m=512] = sum_kb Gp2_kb.T @ msgT_kb  (f32, exact)
                acc = accpool.tile([P, GC * P], mybir.dt.float32, tag="acc")
                accv = acc[:, :].rearrange("q (c p) -> q c p", c=GC)
                for kb in range(KCH):
                    nc.tensor.matmul(
                        accv[:, :, :],
                        gsb[:, kb, :],
                        b[:, :, kb, :],
                        start=(kb == 0),
                        stop=(kb == KCH - 1),
                    )
                acc_tiles[g] = acc

            def emit_post(g):
                a = a_tiles[g]
                acc = acc_tiles.pop(g)
                # evict S to SBUF f32 so the PE can transpose it back
                sev = sevpool.tile([P, GC, P], mybir.dt.float32, tag="sev")
                nc.scalar.copy(sev[:, :, :].rearrange("q c p -> q (c p)"), acc[:, :])
                st = stpool.tile([P, GC, P], mybir.dt.float32, tag="st")
                for c in range(GC):
                    nc.tensor.transpose(st[:, c, :], sev[:, c, :], ident32[:, :])
                # st[p, c, n'] = S[n', c*128+p]; rows (c,p) match a's layout
                ci = cipool.tile([P, GC, NPACK], mybir.dt.int32, tag="ci")
                nc.scalar.copy(ci[:, :, :], st[:, :, :])
                # parity lo = S & 1, hi = (S >> 10) & 1 (bitVec cannot cast)
                elo = epool.tile([P, GC, NPACK], mybir.dt.int32, tag="elo")
                nc.vector.tensor_scalar(
                    elo[:, :, :], ci[:, :, :], 1, None, mybir.AluOpType.bitwise_and
                )
                ehi = epool.tile([P, GC, NPACK], mybir.dt.int32, tag="ehi")
                nc.vector.tensor_scalar(
                    ehi[:, :, :],
                    ci[:, :, :],
                    10,
                    1,
                    mybir.AluOpType.logical_shift_right,
                    mybir.AluOpType.bitwise_and,
                )
                # parity into the output-row tile (0/1 exact in fp16)
                nc.vector.tensor_copy(a[:, :, MSG : MSG + NPACK], elo[:, :, :])
                nc.vector.tensor_copy(a[:, :, MSG + NPACK : MSG + NPAR], ehi[:, :, :])

            def emit_store(g):
                # single cast-store of the full rows: [p, c, 1256] fp16 -> f32
                a = a_tiles.pop(g)
                nc.gpsimd.dma_start(
                    out=out3[g, :, :, :].rearrange("c p k -> p c k"),
                    in_=a[:, :, 0 : MSG + NPAR],
                )

            for g in range(n_groups):
                emit_load(g)
            # stagger: sT/post of group g emitted after T/mm of g+1 so the
            # in-order PE queue never stalls waiting on the ACT sev eviction
            for it in range(n_groups + 1):
                if it < n_groups:
                    emit_transpose(it)
                    emit_mm(it)
                if it >= 1:
                    emit_post(it - 1)
                    emit_store(it - 1)

    nc.compile()
    return nc


def prep_gp(Gp):
    """Pack parity column pairs, pad K to 1024, swizzle to [128, 8*128] fp16."""
    gp = np.asarray(Gp, dtype=np.float32)
    packed = gp[:, :NPACK] + 1024.0 * gp[:, NPACK:]
    gp_pad = np.zeros((KPAD, NPACK), dtype=np.float32)
    gp_pad[:MSG] = packed
    gsw = gp_pad.reshape(KCH, P, NPACK).transpose(1, 0, 2).reshape(P, KCH * NPACK)
    return np.ascontiguousarray(gsw).astype(np.float16)


def kernel(message_bits, Gp):
    global LAST_RESULT
    msg = np.ascontiguousarray(np.asarray(message_bits, dtype=np.float32))
    assert msg.shape == (BATCH, MSG), msg.shape
    gsw = prep_gp(Gp)

    if "nc" not in _CACHE:
        _CACHE["nc"] = build_nc()
    nc = _CACHE["nc"]

    in_maps = [
        {"msg": msg[i * ROWS : (i + 1) * ROWS], "gp": gsw} for i in range(NCORES)
    ]
    res = run_bass_kernel_spmd(
        nc, in_maps, core_ids=list(range(NCORES)), trace=TRACE
    )
    LAST_RESULT = res
    return np.concatenate([r["out"] for r in res.results], axis=0)
